# revision 23
# baseline (speedup 1.0000x reference)
"""Trainium2 Bass kernel for a transformer decoder block (self-attn + cross-attn + MLP).

Sharding (8 NeuronCores):
  - 2 groups of 4 cores; group g handles batch b=g (data parallel on B=2).
  - Within a group, rank r in {0..3} owns head pair (2r, 2r+1) for both attention
    blocks (tensor parallel on H=8) and MLP hidden slice [512r:512(r+1)]
    (tensor parallel on MLP_H=2048).
  - LayerNorms are computed replicated (full T) on every core.
  - Attention head outputs are exchanged with a small bf16 AllGather (512KB/rank);
    each core then runs the output projection itself (cheap, full T).
  - The MLP second matmul produces partial sums; the residual stream is folded in
    on rank 0 of each group, and one ReduceScatter(add) both sums the partials and
    token-shards the final output: core (g, r) emits output tokens [512r:512(r+1)]
    of batch g. The host reassembles the full [B, T, D] output.

All matmuls run in bf16 (fp32 accumulation); LayerNorm statistics, softmax
normalization and the residual stream stay in fp32. Softmax skips the max
subtraction (scores are O(1) for these scales) and applies masks multiplicatively
after exp. Host-side specialization (legal: the program is compiled per call):
  - tril self-mask -> causal block skipping + on-chip affine triangle masks
  - all-ones masks -> no masking
  - all-zero biases / unit LayerNorm gains -> skipped
  - otherwise a general path applies masks / biases / gains from extra inputs.
"""

import numpy as np
import ml_dtypes

import concourse.bass as bass
import concourse.mybir as mybir
import concourse.tile as tile
from concourse import bacc
from concourse.bass_utils import run_bass_kernel_spmd
from concourse.masks import make_identity

B, T, S, D, H = 2, 2048, 2048, 512, 8
HD = D // H          # 64
MLP_H = 4 * D        # 2048
EPS = 1e-5
P = 128
NB = T // P          # 16 token blocks
KT = D // P          # 4 contraction tiles over D
NCH = T // 512       # 4 query/token 512-chunks
G = 4                # cores per group
HL = 2 * HD          # 128 local head columns (2 heads)
HIDL = MLP_H // G    # 512 local mlp hidden
F32 = mybir.dt.float32
BF16 = mybir.dt.bfloat16
BF16_NP = ml_dtypes.bfloat16

_cache = {}
DEBUG_TAPS = False  # when True, _build adds intermediate tensors as extra outputs
LAST_RESULTS = None


def _build(cfg):
    """Build the (SPMD-identical) Bass program for one core. cfg is a tuple of
    specialization flags."""
    (self_mode, cross_mode, apply_gb1, apply_gb2, apply_gb3,
     qb_nz, kb_nz, vb_nz, sab_nz, cqb_nz, ckb_nz, cvb_nz, cab_nz,
     b1_nz, b2_nz) = cfg

    nc = bacc.Bacc("TRN2", debug=False, num_devices=8)

    # ---------------- I/O -----------------
    x_d = nc.dram_tensor("x", [T, D], F32, kind="ExternalInput")
    encT_d = nc.dram_tensor("encT", [D, S], BF16, kind="ExternalInput")
    wqkv_d = nc.dram_tensor("wqkv", [D, 3 * HL], BF16, kind="ExternalInput")
    wsa_d = nc.dram_tensor("wsa", [D, D], BF16, kind="ExternalInput")
    wcq_d = nc.dram_tensor("wcq", [D, HL], BF16, kind="ExternalInput")
    wckv_d = nc.dram_tensor("wckv", [D, 2 * HL], BF16, kind="ExternalInput")
    wca_d = nc.dram_tensor("wca", [D, D], BF16, kind="ExternalInput")
    w1_d = nc.dram_tensor("w1", [D, HIDL], BF16, kind="ExternalInput")
    w2_d = nc.dram_tensor("w2", [HIDL, D], BF16, kind="ExternalInput")
    gate_d = nc.dram_tensor("gate", [1, 1], F32, kind="ExternalInput")
    out_d = nc.dram_tensor("out", [T // G, D], F32, kind="ExternalOutput")

    maskT_self_d = maskT_cross_d = None
    if self_mode == "general":
        maskT_self_d = nc.dram_tensor("maskT_self", [T, T], BF16, kind="ExternalInput")
    if cross_mode == "general":
        maskT_cross_d = nc.dram_tensor("maskT_cross", [S, T], BF16, kind="ExternalInput")

    # general-path params (broadcast rows in DRAM -> [128, N] SBUF via step-0 AP)
    def opt_in(name, shape, dt=F32, cond=True):
        return nc.dram_tensor(name, shape, dt, kind="ExternalInput") if cond else None

    g1_d = opt_in("ln1_g", [D], cond=apply_gb1)
    b1ln_d = opt_in("ln1_b", [D], cond=apply_gb1)
    g2_d = opt_in("ln2_g", [D], cond=apply_gb2)
    b2ln_d = opt_in("ln2_b", [D], cond=apply_gb2)
    g3_d = opt_in("ln3_g", [D], cond=apply_gb3)
    b3ln_d = opt_in("ln3_b", [D], cond=apply_gb3)
    qb_d = opt_in("q_bias", [HL], cond=qb_nz)      # per-partition column
    kb_d = opt_in("k_bias", [HL], cond=kb_nz)
    vb_d = opt_in("v_bias", [HL], cond=vb_nz)      # per-free broadcast
    sab_d = opt_in("sa_bias", [D], cond=sab_nz)
    cqb_d = opt_in("cq_bias", [HL], cond=cqb_nz)
    ckb_d = opt_in("ck_bias", [HL], cond=ckb_nz)
    cvb_d = opt_in("cv_bias", [HL], cond=cvb_nz)
    cab_d = opt_in("ca_bias", [D], cond=cab_nz)
    mb1_d = opt_in("mlp_b1", [HIDL], cond=b1_nz)
    mb2_d = opt_in("mlp_b2_gated", [D], cond=b2_nz)

    with tile.TileContext(nc) as tc:
        const = tc.alloc_tile_pool(name="const", bufs=1)
        xres = tc.alloc_tile_pool(name="xres", bufs=1)
        big = tc.alloc_tile_pool(name="big", bufs=1)
        work = tc.alloc_tile_pool(name="work", bufs=3)
        probs_p = tc.alloc_tile_pool(name="probs", bufs=3)
        dram = tc.alloc_tile_pool(name="dram", bufs=1, space="DRAM")
        psS = tc.alloc_tile_pool(name="psS", bufs=2, space="PSUM")
        psB = tc.alloc_tile_pool(name="psB", bufs=1, space="PSUM")
        psG = tc.alloc_tile_pool(name="psG", bufs=2, space="PSUM")

        sync = nc.sync

        def bcast_row(dram_ap, n, dt=F32, parts=P):
            """[n] DRAM -> [parts, n] SBUF, replicated across partitions."""
            t = const.tile([parts, n], dt)
            src = bass.AP(tensor=dram_ap.tensor, offset=dram_ap.offset,
                          ap=[[0, parts]] + list(dram_ap.ap))
            nc.gpsimd.dma_start(out=t, in_=src)
            return t

        def col_vec(dram_ap, n):
            """[n] DRAM -> [n, 1] SBUF column (per-partition scalar)."""
            t = const.tile([n, 1], F32)
            sync.dma_start(out=t, in_=dram_ap.rearrange("n -> n 1"))
            return t

        # ---------------- constants / weights -----------------
        ident = const.tile([P, P], BF16)
        make_identity(nc, ident)
        eps_sb = const.tile([P, 1], F32)
        nc.vector.memset(eps_sb, EPS)
        gate_sb = const.tile([P, 1], F32)
        nc.gpsimd.dma_start(out=gate_sb, in_=bass.AP(
            tensor=gate_d.ap().tensor, offset=0, ap=[[0, P], [1, 1]]))

        def load_w(d, n):
            t = const.tile([P, KT, n], BF16)
            sync.dma_start(out=t, in_=d.ap().rearrange("(k p) n -> p k n", p=P))
            return t

        wqkv_sb = load_w(wqkv_d, 3 * HL)
        wsa_sb = load_w(wsa_d, D)
        wcq_sb = load_w(wcq_d, HL)
        wckv_sb = load_w(wckv_d, 2 * HL)
        wca_sb = load_w(wca_d, D)
        w1_sb = load_w(w1_d, HIDL)
        w2_sb = const.tile([P, HIDL // P, D], BF16)
        sync.dma_start(out=w2_sb, in_=w2_d.ap().rearrange("(k p) n -> p k n", p=P))

        encT_sb = const.tile([P, KT, S], BF16)
        sync.dma_start(out=encT_sb, in_=encT_d.ap().rearrange("(k p) t -> p k t", p=P))

        x_sb = xres.tile([P, NB, D], F32)
        sync.dma_start(out=x_sb, in_=x_d.ap().rearrange("(n p) d -> p n d", p=P))

        gb = {}
        for nm, gd, bd, ap_f in (("ln1", g1_d, b1ln_d, apply_gb1),
                                 ("ln2", g2_d, b2ln_d, apply_gb2),
                                 ("ln3", g3_d, b3ln_d, apply_gb3)):
            if ap_f:
                gb[nm] = (bcast_row(gd.ap(), D), bcast_row(bd.ap(), D))
        qb_sb = col_vec(qb_d.ap(), HL) if qb_nz else None
        kb_sb = col_vec(kb_d.ap(), HL) if kb_nz else None
        cqb_sb = col_vec(cqb_d.ap(), HL) if cqb_nz else None
        ckb_sb = col_vec(ckb_d.ap(), HL) if ckb_nz else None
        vb_sb = bcast_row(vb_d.ap(), HL) if vb_nz else None
        cvb_sb = bcast_row(cvb_d.ap(), HL) if cvb_nz else None
        sab_sb = bcast_row(sab_d.ap(), D) if sab_nz else None
        cab_sb = bcast_row(cab_d.ap(), D) if cab_nz else None
        mb1_sb = col_vec(mb1_d.ap(), HIDL) if b1_nz else None  # [512,1] -> use [:, hch]
        mb1_cols = None
        if b1_nz:
            mb1_cols = const.tile([P, HIDL // P], F32)
            sync.dma_start(out=mb1_cols,
                           in_=mb1_d.ap().rearrange("(k p) -> p k", p=P))
        mb2_sb = bcast_row(mb2_d.ap(), D) if b2_nz else None

        tap_ctr = [0]

        def tap(name, ap):
            if not DEBUG_TAPS:
                return
            d = nc.dram_tensor(f"dbg_{name}", list(ap.shape), ap.dtype,
                               kind="ExternalOutput")
            sync.dma_start(out=d.ap(), in_=ap)

        # ---------------- helpers -----------------
        def layernorm_transpose(ln_name, out_xT, n_blocks=NB):
            """LN(x_sb) (token-major stats) -> bf16 -> transpose into out_xT [P, KT, T]."""
            gbp = gb.get(ln_name)
            for blk in range(n_blocks):
                stats = work.tile([P, 6], F32, tag="stats")
                nc.vector.bn_stats(out=stats, in_=x_sb[:, blk])
                mv = work.tile([P, 2], F32, tag="mv")
                nc.vector.bn_aggr(out=mv, in_=stats)
                # rstd = sqrt(1/(var+eps)); DVE approx reciprocal avoids the
                # walrus sync-wait limit on InstReciprocal.
                rr = work.tile([P, 1], F32, tag="rr")
                nc.vector.tensor_scalar(out=rr, in0=mv[:, 1:2], scalar1=float(EPS),
                                        scalar2=None, op0=mybir.AluOpType.add)
                nc.vector.reciprocal_approx_fast(out=mv[:, 1:2], in_=rr)
                nc.scalar.activation(out=mv[:, 1:2], in_=mv[:, 1:2],
                                     func=mybir.ActivationFunctionType.Sqrt,
                                     scale=1.0)
                xn = work.tile([P, D], BF16, tag="xnorm")
                if gbp is None:
                    nc.vector.tensor_scalar(out=xn, in0=x_sb[:, blk],
                                            scalar1=mv[:, 0:1], scalar2=mv[:, 1:2],
                                            op0=mybir.AluOpType.subtract,
                                            op1=mybir.AluOpType.mult)
                else:
                    xf = work.tile([P, D], F32, tag="xnorm_f")
                    nc.vector.tensor_scalar(out=xf, in0=x_sb[:, blk],
                                            scalar1=mv[:, 0:1], scalar2=mv[:, 1:2],
                                            op0=mybir.AluOpType.subtract,
                                            op1=mybir.AluOpType.mult)
                    nc.vector.tensor_mul(out=xf, in0=xf, in1=gbp[0])
                    nc.vector.tensor_add(out=xn, in0=xf, in1=gbp[1])
                pst = psG.tile([P, D], BF16, tag="psG")
                for kt in range(KT):
                    nc.tensor.transpose(pst[:, kt * P:(kt + 1) * P],
                                        xn[:, kt * P:(kt + 1) * P], ident)
                nc.vector.tensor_copy(
                    out=out_xT[:, :, blk * P:(blk + 1) * P],
                    in_=pst.rearrange("p (k t) -> p k t", k=KT))

        def mm_TN(out_sb, w_sb, w_col0, w_cols, rhs_T, bias_col=None):
            """out_sb[M=w_cols rows, T] (bf16) = w[:, w_col0:+w_cols].T @ rhs_T.
            Contracts over D (KT tiles)."""
            for nch in range(NCH):
                ps = psG.tile([P, 512], F32, tag="psG")
                for kt in range(KT):
                    nc.tensor.matmul(ps[:w_cols], w_sb[:, kt, w_col0:w_col0 + w_cols],
                                     rhs_T[:, kt, nch * 512:(nch + 1) * 512],
                                     start=(kt == 0), stop=(kt == KT - 1))
                if bias_col is None:
                    nc.vector.tensor_copy(out=out_sb[:w_cols, nch * 512:(nch + 1) * 512],
                                          in_=ps[:w_cols])
                else:
                    nc.vector.tensor_scalar(out=out_sb[:w_cols, nch * 512:(nch + 1) * 512],
                                            in0=ps[:w_cols], scalar1=bias_col,
                                            scalar2=None, op0=mybir.AluOpType.add)

        def mm_val(v_sb, src_T, w_sb, w_col0, bias_b=None):
            """v_sb [P, NB, 130] token-major values (+ones cols) = src.T @ w[:, w_col0:+128]."""
            nc.vector.memset(
                v_sb.rearrange("p n (two c) -> p n two c", two=2)[:, :, :, HD:HD + 1], 1.0)
            for blk in range(NB):
                ps = psG.tile([P, HL], F32, tag="psG")
                for kt in range(KT):
                    nc.tensor.matmul(ps, src_T[:, kt, blk * P:(blk + 1) * P],
                                     w_sb[:, kt, w_col0:w_col0 + HL],
                                     start=(kt == 0), stop=(kt == KT - 1))
                dst = v_sb[:, blk].rearrange("p (two c) -> p two c", two=2)[:, :, :HD]
                src = ps.rearrange("p (two c) -> p two c", two=2)
                if bias_b is None:
                    nc.vector.tensor_copy(out=dst, in_=src)
                else:
                    bb = bias_b.rearrange("p (two c) -> p two c", two=2)
                    nc.vector.tensor_add(out=dst, in0=src, in1=bb)

        def attention(qT, kT, v_sb, attnT_loc, mode, maskT_d, n_kb=NB):
            """attnT_loc [P(2 heads*64), T] bf16 = softmax(qk^T/8, mask) @ v, transposed.
            mode: 'causal' | 'ones' | 'general'."""
            for h in range(2):
                qh = qT[h * HD:(h + 1) * HD]
                kh = kT[h * HD:(h + 1) * HD]
                avT = psB.tile([P, NCH, 512], F32, tag="avT")
                for kb in range(n_kb):
                    if mode == "causal":
                        qcs = [qc for qc in range(NCH) if 4 * qc + 3 >= kb]
                    else:
                        qcs = list(range(NCH))
                    pr = probs_p.tile([P, T], BF16, tag="probs")
                    if mode == "general":
                        m_sb = probs_p.tile([P, T], BF16, tag="mask")
                        sync.dma_start(out=m_sb, in_=maskT_d.ap()[kb * P:(kb + 1) * P, :])
                    for qc in qcs:
                        ps = psS.tile([P, 512], F32, tag="scores")
                        nc.tensor.matmul(ps, kh[:, kb * P:(kb + 1) * P],
                                         qh[:, qc * 512:(qc + 1) * 512],
                                         start=True, stop=True)
                        nc.scalar.activation(out=pr[:, qc * 512:(qc + 1) * 512], in_=ps,
                                             func=mybir.ActivationFunctionType.Exp,
                                             scale=float(HD) ** -0.5)
                        if mode == "causal" and kb // 4 == qc:
                            j = kb % 4
                            # keep prob where key k <= query q: (q - k - 128j) >= 0
                            nc.gpsimd.affine_select(
                                out=pr[:, qc * 512:(qc + 1) * 512],
                                in_=pr[:, qc * 512:(qc + 1) * 512],
                                pattern=[[1, 512]], channel_multiplier=-1,
                                base=-128 * j, compare_op=mybir.AluOpType.is_ge,
                                fill=0.0)
                        elif mode == "general":
                            nc.vector.tensor_mul(out=pr[:, qc * 512:(qc + 1) * 512],
                                                 in0=pr[:, qc * 512:(qc + 1) * 512],
                                                 in1=m_sb[:, qc * 512:(qc + 1) * 512])
                    for qc in qcs:
                        last_kb = min(n_kb - 1, 4 * qc + 3) if mode == "causal" else n_kb - 1
                        nc.tensor.matmul(avT[:HD + 1, qc],
                                         v_sb[:, kb, h * (HD + 1):(h + 1) * (HD + 1)],
                                         pr[:, qc * 512:(qc + 1) * 512],
                                         start=(kb == 0), stop=(kb == last_kb))
                rec = work.tile([HD + 1, T], F32, tag="rec")
                nc.vector.tensor_copy(out=rec[HD:HD + 1],
                                      in_=avT[HD:HD + 1].rearrange("p a b -> p (a b)"))
                # broadcast the sums row to partitions 0..63 via a DRAM bounce
                # (step-0 partition APs are only legal on DRAM).
                rec_d = dram.tile([1, T], F32, tag="rec_d")
                sync.dma_start(out=rec_d, in_=rec[HD:HD + 1])
                src = bass.AP(tensor=rec_d.tensor, offset=rec_d.offset,
                              ap=[[0, HD]] + list(rec_d.ap)[1:])
                sync.dma_start(out=rec[:HD], in_=src)
                rec2 = work.tile([HD, T], F32, tag="rec2")
                nc.vector.reciprocal(out=rec2, in_=rec[:HD])
                if DEBUG_TAPS and h == 0:
                    tap_ctr[0] += 1
                    tap(f"sums{tap_ctr[0]}", rec[HD:HD + 1])
                    tap(f"rec{tap_ctr[0]}", rec2)
                nc.vector.tensor_mul(
                    out=attnT_loc[h * HD:(h + 1) * HD].rearrange("p (a b) -> p a b", a=NCH),
                    in0=avT[:HD],
                    in1=rec2.rearrange("p (a b) -> p a b", a=NCH))

        def proj_residual(attnT_full, w_sb, bias_row):
            """x_sb += attnT_full.T @ w (+bias)."""
            for blk in range(NB):
                ps = psG.tile([P, D], F32, tag="psG")
                for kt in range(KT):
                    nc.tensor.matmul(ps, attnT_full[:, kt, blk * P:(blk + 1) * P],
                                     w_sb[:, kt, :], start=(kt == 0), stop=(kt == KT - 1))
                nc.vector.tensor_add(out=x_sb[:, blk], in0=x_sb[:, blk], in1=ps)
                if bias_row is not None:
                    nc.vector.tensor_add(out=x_sb[:, blk], in0=x_sb[:, blk], in1=bias_row)

        # ================ pipeline ================
        xT = big.tile([P, KT, T], BF16, tag="xT", name="x1T")
        layernorm_transpose("ln1", xT)
        tap("x1T", xT)

        qT = big.tile([P, T], BF16, tag="qT", name="qT_self")
        kT = big.tile([P, T], BF16, tag="kT", name="kT_self")
        v_sb = big.tile([P, NB, 2 * (HD + 1)], BF16, tag="v", name="v_self")
        mm_TN(qT, wqkv_sb, 0, HL, xT, qb_sb)
        mm_TN(kT, wqkv_sb, HL, HL, xT, kb_sb)
        mm_val(v_sb, xT, wqkv_sb, 2 * HL, vb_sb)
        tap("qT", qT)
        tap("kT", kT)
        tap("v", v_sb)

        attnT_loc = big.tile([P, T], BF16, tag="attnT", name="attnT_sa")
        attention(qT, kT, v_sb, attnT_loc, self_mode, maskT_self_d)
        tap("attnT_sa", attnT_loc)

        # AllGather self-attention heads (bf16)
        ag1_in = dram.tile([P, T], BF16, name="ag1_in")
        ag1_out = dram.tile([G * P, T], BF16, name="ag1_out")
        sync.dma_start(out=ag1_in, in_=attnT_loc)
        nc.gpsimd.collective_compute(
            "AllGather", mybir.AluOpType.bypass,
            replica_groups=[[0, 1, 2, 3], [4, 5, 6, 7]],
            ins=[ag1_in.opt()], outs=[ag1_out.opt()])
        attnT_full = big.tile([P, KT, T], BF16, tag="attnT_full", name="attnT_sa_full")
        sync.dma_start(out=attnT_full,
                       in_=ag1_out.rearrange("(k p) t -> p k t", p=P))
        proj_residual(attnT_full, wsa_sb, sab_sb)
        tap("x_after_sa", x_sb)

        # ---- cross attention ----
        layernorm_transpose("ln2", xT)  # xT now holds x2T
        mm_TN(qT, wcq_sb, 0, HL, xT, cqb_sb)
        mm_TN(kT, wckv_sb, 0, HL, encT_sb, ckb_sb)
        mm_val(v_sb, encT_sb, wckv_sb, HL, cvb_sb)
        attention(qT, kT, v_sb, attnT_loc, cross_mode, maskT_cross_d, n_kb=S // P)

        ag2_in = dram.tile([P, T], BF16, name="ag2_in")
        ag2_out = dram.tile([G * P, T], BF16, name="ag2_out")
        sync.dma_start(out=ag2_in, in_=attnT_loc)
        nc.gpsimd.collective_compute(
            "AllGather", mybir.AluOpType.bypass,
            replica_groups=[[0, 1, 2, 3], [4, 5, 6, 7]],
            ins=[ag2_in.opt()], outs=[ag2_out.opt()])
        sync.dma_start(out=attnT_full,
                       in_=ag2_out.rearrange("(k p) t -> p k t", p=P))
        proj_residual(attnT_full, wca_sb, cab_sb)
        tap("x_after_ca", x_sb)

        # ---- MLP (hidden-slice tensor parallel) ----
        layernorm_transpose("ln3", xT)  # xT now holds x3T
        hT = big.tile([P, HIDL // P, T], BF16, tag="hT", name="hT")
        for hch in range(HIDL // P):
            for nch in range(NCH):
                ps = psG.tile([P, 512], F32, tag="psG")
                for kt in range(KT):
                    nc.tensor.matmul(ps, w1_sb[:, kt, hch * P:(hch + 1) * P],
                                     xT[:, kt, nch * 512:(nch + 1) * 512],
                                     start=(kt == 0), stop=(kt == KT - 1))
                nc.scalar.activation(
                    out=hT[:, hch, nch * 512:(nch + 1) * 512], in_=ps,
                    func=mybir.ActivationFunctionType.Gelu,
                    bias=(mb1_cols[:, hch:hch + 1] if b1_nz else 0.0), scale=1.0)

        rs_in = dram.tile([T, D], F32, name="rs_in")
        rs_out = dram.tile([T // G, D], F32, name="rs_out")
        for blk in range(NB):
            ps = psG.tile([P, D], F32, tag="psG")
            for hch in range(HIDL // P):
                nc.tensor.matmul(ps, hT[:, hch, blk * P:(blk + 1) * P],
                                 w2_sb[:, hch, :], start=(hch == 0),
                                 stop=(hch == HIDL // P - 1))
            part = work.tile([P, D], F32, tag="part")
            nc.vector.scalar_tensor_tensor(out=part, in0=x_sb[:, blk], scalar=gate_sb[:, 0:1],
                                           in1=ps, op0=mybir.AluOpType.mult,
                                           op1=mybir.AluOpType.add)
            if b2_nz:
                nc.vector.tensor_add(out=part, in0=part, in1=mb2_sb)
            sync.dma_start(out=rs_in[blk * P:(blk + 1) * P, :], in_=part)

        nc.gpsimd.collective_compute(
            "ReduceScatter", mybir.AluOpType.add,
            replica_groups=[[0, 1, 2, 3], [4, 5, 6, 7]],
            ins=[rs_in.opt()], outs=[rs_out.opt()])
        sync.dma_start(out=out_d.ap(), in_=rs_out)

        for p in reversed((const, xres, big, work, probs_p, dram, psS, psB, psG)):
            p.release()

    nc.compile()
    return nc


def prepare(inputs):
    """Host-side prep: specialization flags, program build, per-core input maps.
    Returns (nc, in_maps)."""
    x = np.asarray(inputs["x"], np.float32)
    enc = np.asarray(inputs["encoder_out"], np.float32)
    self_mask = np.asarray(inputs["self_mask"]).astype(bool)[0, 0]
    cross_mask = np.asarray(inputs["cross_mask"]).astype(bool)[0, 0]
    qkv_w = np.asarray(inputs["qkv_w"], np.float32)
    qkv_b = np.asarray(inputs["qkv_b"], np.float32)
    sa_proj_w = np.asarray(inputs["sa_proj_w"], np.float32)
    sa_proj_b = np.asarray(inputs["sa_proj_b"], np.float32)
    ln1_g = np.asarray(inputs["ln1_g"], np.float32)
    ln1_b = np.asarray(inputs["ln1_b"], np.float32)
    q_w = np.asarray(inputs["q_w"], np.float32)
    q_b = np.asarray(inputs["q_b"], np.float32)
    kv_w = np.asarray(inputs["kv_w"], np.float32)
    kv_b = np.asarray(inputs["kv_b"], np.float32)
    ca_proj_w = np.asarray(inputs["ca_proj_w"], np.float32)
    ca_proj_b = np.asarray(inputs["ca_proj_b"], np.float32)
    ln2_g = np.asarray(inputs["ln2_g"], np.float32)
    ln2_b = np.asarray(inputs["ln2_b"], np.float32)
    mlp_w1 = np.asarray(inputs["mlp_w1"], np.float32)
    mlp_b1 = np.asarray(inputs["mlp_b1"], np.float32)
    mlp_w2 = np.asarray(inputs["mlp_w2"], np.float32)
    mlp_b2 = np.asarray(inputs["mlp_b2"], np.float32)
    ln3_g = np.asarray(inputs["ln3_g"], np.float32)
    ln3_b = np.asarray(inputs["ln3_b"], np.float32)

    def mask_mode(m):
        if m.all():
            return "ones"
        if np.array_equal(m, np.tril(np.ones(m.shape, bool))):
            return "causal"
        return "general"

    self_mode = mask_mode(self_mask)
    cross_mode = mask_mode(cross_mask)
    if cross_mode == "causal":  # causal path only wired for the self block
        cross_mode = "general"

    def nz(a):
        return bool(np.any(a != 0.0))

    def nontriv(g, b):
        return bool(np.any(g != 1.0) or np.any(b != 0.0))

    cfg = (self_mode, cross_mode,
           nontriv(ln1_g, ln1_b), nontriv(ln2_g, ln2_b), nontriv(ln3_g, ln3_b),
           nz(qkv_b[:D]), nz(qkv_b[D:2 * D]), nz(qkv_b[2 * D:]),
           nz(sa_proj_b), nz(q_b), nz(kv_b[:D]), nz(kv_b[D:]), nz(ca_proj_b),
           nz(mlp_b1), nz(mlp_b2))

    if cfg not in _cache:
        _cache[cfg] = _build(cfg)
    nc = _cache[cfg]

    bf = lambda a: np.ascontiguousarray(a.astype(BF16_NP))
    in_maps = []
    for core in range(8):
        g, r = divmod(core, G)
        hc = slice(r * HL, (r + 1) * HL)       # this core's 128 head columns
        hid = slice(r * HIDL, (r + 1) * HIDL)  # this core's mlp hidden slice
        m = {
            "x": np.ascontiguousarray(x[g]),
            "encT": bf(enc[g].T),
            "wqkv": bf(np.concatenate(
                [qkv_w[:, hc], qkv_w[:, D:][:, hc], qkv_w[:, 2 * D:][:, hc]], axis=1)),
            "wsa": bf(sa_proj_w),
            "wcq": bf(q_w[:, hc]),
            "wckv": bf(np.concatenate([kv_w[:, :D][:, hc], kv_w[:, D:][:, hc]], axis=1)),
            "wca": bf(ca_proj_w),
            "w1": bf(mlp_w1[:, hid]),
            "w2": bf(mlp_w2[hid, :]),
            "gate": np.full((1, 1), 1.0 if r == 0 else 0.0, np.float32),
        }
        if self_mode == "general":
            m["maskT_self"] = bf(self_mask.T.astype(np.float32))
        if cross_mode == "general":
            m["maskT_cross"] = bf(cross_mask.T.astype(np.float32))
        if cfg[2]:
            m["ln1_g"], m["ln1_b"] = ln1_g, ln1_b
        if cfg[3]:
            m["ln2_g"], m["ln2_b"] = ln2_g, ln2_b
        if cfg[4]:
            m["ln3_g"], m["ln3_b"] = ln3_g, ln3_b
        if cfg[5]:
            m["q_bias"] = np.ascontiguousarray(qkv_b[:D][hc])
        if cfg[6]:
            m["k_bias"] = np.ascontiguousarray(qkv_b[D:2 * D][hc])
        if cfg[7]:
            m["v_bias"] = np.ascontiguousarray(qkv_b[2 * D:][hc])
        if cfg[8]:
            m["sa_bias"] = sa_proj_b
        if cfg[9]:
            m["cq_bias"] = np.ascontiguousarray(q_b[hc])
        if cfg[10]:
            m["ck_bias"] = np.ascontiguousarray(kv_b[:D][hc])
        if cfg[11]:
            m["cv_bias"] = np.ascontiguousarray(kv_b[D:][hc])
        if cfg[12]:
            m["ca_bias"] = ca_proj_b
        if cfg[13]:
            m["mlp_b1"] = np.ascontiguousarray(mlp_b1[hid])
        if cfg[14]:
            m["mlp_b2_gated"] = mlp_b2 if r == 0 else np.zeros_like(mlp_b2)
        in_maps.append(m)

    return nc, in_maps


def assemble(results) -> np.ndarray:
    out = np.empty((B, T, D), np.float32)
    for core in range(8):
        g, r = divmod(core, G)
        out[g, r * (T // G):(r + 1) * (T // G), :] = results[core]["out"]
    return out


def kernel(**inputs) -> np.ndarray:
    nc, in_maps = prepare(inputs)
    res = run_bass_kernel_spmd(nc, in_maps, core_ids=list(range(8)))
    global LAST_RESULTS
    LAST_RESULTS = res
    return assemble(res.results)


# revision 43
# speedup vs baseline: 263.8325x; 263.8325x over previous
"""Trainium2 Bass kernel for a transformer decoder block (self-attn + cross-attn + MLP).

Sharding (8 NeuronCores):
  - 2 groups of 4 cores; group g handles batch b=g (data parallel on B=2).
  - Within a group, rank r in {0..3} owns head pair (2r, 2r+1) for both attention
    blocks (tensor parallel on H=8) and MLP hidden slice [512r:512(r+1)]
    (tensor parallel on MLP_H=2048).
  - LayerNorms are computed replicated (full T) on every core.
  - Attention head outputs are exchanged with a small bf16 AllGather (512KB/rank);
    each core then runs the output projection itself (cheap, full T).
  - The MLP second matmul produces partial sums; the residual stream is folded in
    on rank 0 of each group, and one ReduceScatter(add) both sums the partials and
    token-shards the final output: core (g, r) emits output tokens [512r:512(r+1)]
    of batch g. The host reassembles the full [B, T, D] output.

All matmuls run in bf16 (fp32 accumulation); LayerNorm statistics, softmax
normalization and the residual stream stay in fp32. Softmax skips the max
subtraction (scores are O(1) for these scales) and applies masks multiplicatively
after exp. Host-side specialization (legal: the program is compiled per call):
  - tril self-mask -> causal block skipping + on-chip affine triangle masks
  - all-ones masks -> no masking
  - all-zero biases / unit LayerNorm gains -> skipped
  - otherwise a general path applies masks / biases / gains from extra inputs.
"""

import numpy as np
import ml_dtypes

import concourse.bass as bass
import concourse.mybir as mybir
import concourse.tile as tile
from concourse import bacc
from concourse.bass_utils import run_bass_kernel_spmd
from concourse.masks import make_identity

B, T, S, D, H = 2, 2048, 2048, 512, 8
HD = D // H          # 64
MLP_H = 4 * D        # 2048
EPS = 1e-5
P = 128
NB = T // P          # 16 token blocks
KT = D // P          # 4 contraction tiles over D
NCH = T // 512       # 4 query/token 512-chunks
G = 4                # cores per group
HL = 2 * HD          # 128 local head columns (2 heads)
HIDL = MLP_H // G    # 512 local mlp hidden
F32 = mybir.dt.float32
BF16 = mybir.dt.bfloat16
BF16_NP = ml_dtypes.bfloat16

_cache = {}
DEBUG_TAPS = False  # when True, _build adds intermediate tensors as extra outputs
LAST_RESULTS = None


def _build(cfg, sim1=False):
    """Build the (SPMD-identical) Bass program for one core. cfg is a tuple of
    specialization flags. sim1=True replaces collectives with equivalent-byte
    local DMA copies so TimelineSim (single-core, no collectives) can run."""
    (self_mode, cross_mode, apply_gb1, apply_gb2, apply_gb3,
     qb_nz, kb_nz, vb_nz, sab_nz, cqb_nz, ckb_nz, cvb_nz, cab_nz,
     b1_nz, b2_nz) = cfg

    nc = bacc.Bacc("TRN2", debug=False, num_devices=8)

    # ---------------- I/O -----------------
    x_d = nc.dram_tensor("x", [T, D], F32, kind="ExternalInput")
    encT_d = nc.dram_tensor("encT", [D, S], BF16, kind="ExternalInput")
    wqkv_d = nc.dram_tensor("wqkv", [D, 3 * HL], BF16, kind="ExternalInput")
    wsa_d = nc.dram_tensor("wsa", [D, D], BF16, kind="ExternalInput")
    wcq_d = nc.dram_tensor("wcq", [D, HL], BF16, kind="ExternalInput")
    wckv_d = nc.dram_tensor("wckv", [D, 2 * HL], BF16, kind="ExternalInput")
    wca_d = nc.dram_tensor("wca", [D, D], BF16, kind="ExternalInput")
    w1_d = nc.dram_tensor("w1", [D, HIDL], BF16, kind="ExternalInput")
    w2_d = nc.dram_tensor("w2", [HIDL, D], BF16, kind="ExternalInput")
    gate_d = nc.dram_tensor("gate", [1, 1], F32, kind="ExternalInput")
    out_d = nc.dram_tensor("out", [T // G, D], F32, kind="ExternalOutput")

    maskT_self_d = maskT_cross_d = None
    if self_mode == "general":
        maskT_self_d = nc.dram_tensor("maskT_self", [T, T], BF16, kind="ExternalInput")
    if cross_mode == "general":
        maskT_cross_d = nc.dram_tensor("maskT_cross", [S, T], BF16, kind="ExternalInput")

    # general-path params (broadcast rows in DRAM -> [128, N] SBUF via step-0 AP)
    def opt_in(name, shape, dt=F32, cond=True):
        return nc.dram_tensor(name, shape, dt, kind="ExternalInput") if cond else None

    g1_d = opt_in("ln1_g", [D], cond=apply_gb1)
    b1ln_d = opt_in("ln1_b", [D], cond=apply_gb1)
    g2_d = opt_in("ln2_g", [D], cond=apply_gb2)
    b2ln_d = opt_in("ln2_b", [D], cond=apply_gb2)
    g3_d = opt_in("ln3_g", [D], cond=apply_gb3)
    b3ln_d = opt_in("ln3_b", [D], cond=apply_gb3)
    qb_d = opt_in("q_bias", [HL], cond=qb_nz)      # per-partition column
    kb_d = opt_in("k_bias", [HL], cond=kb_nz)
    vb_d = opt_in("v_bias", [HL], cond=vb_nz)      # per-free broadcast
    sab_d = opt_in("sa_bias", [D], cond=sab_nz)
    cqb_d = opt_in("cq_bias", [HL], cond=cqb_nz)
    ckb_d = opt_in("ck_bias", [HL], cond=ckb_nz)
    cvb_d = opt_in("cv_bias", [HL], cond=cvb_nz)
    cab_d = opt_in("ca_bias", [D], cond=cab_nz)
    mb1_d = opt_in("mlp_b1", [HIDL], cond=b1_nz)
    mb2_d = opt_in("mlp_b2_gated", [D], cond=b2_nz)

    with tile.TileContext(nc) as tc:
        const = tc.alloc_tile_pool(name="const", bufs=1)
        xres = tc.alloc_tile_pool(name="xres", bufs=1)
        big = tc.alloc_tile_pool(name="big", bufs=1)
        work = tc.alloc_tile_pool(name="work", bufs=3)
        probs_p = tc.alloc_tile_pool(name="probs", bufs=3)
        dram = tc.alloc_tile_pool(name="dram", bufs=1, space="DRAM")
        # PSUM budget (8 banks): "ps" [128,1024]f32 x2 bufs = 4 banks shared by
        # scores and all general matmul outputs; "avT" [65,2,512] x2 bufs = 4.
        psG = tc.alloc_tile_pool(name="psG", bufs=2, space="PSUM")
        psB = tc.alloc_tile_pool(name="psB", bufs=2, space="PSUM")

        sync = nc.sync

        def bcast_row(dram_ap, n, dt=F32, parts=P):
            """[n] DRAM -> [parts, n] SBUF, replicated across partitions."""
            t = const.tile([parts, n], dt, tag=f"bc_{dram_ap.tensor.name}",
                           name=f"bc_{dram_ap.tensor.name}")
            src = bass.AP(tensor=dram_ap.tensor, offset=dram_ap.offset,
                          ap=[[0, parts]] + list(dram_ap.ap))
            nc.gpsimd.dma_start(out=t, in_=src)
            return t

        def col_vec(dram_ap, n):
            """[n] DRAM -> [n, 1] SBUF column (per-partition scalar)."""
            t = const.tile([n, 1], F32, tag=f"cv_{dram_ap.tensor.name}",
                           name=f"cv_{dram_ap.tensor.name}")
            sync.dma_start(out=t, in_=dram_ap.rearrange("n -> n 1"))
            return t

        # ---------------- constants / weights -----------------
        ident = const.tile([P, P], BF16)
        make_identity(nc, ident)
        eps_sb = const.tile([P, 1], F32)
        nc.vector.memset(eps_sb, EPS)
        gate_sb = const.tile([P, 1], F32)
        nc.gpsimd.dma_start(out=gate_sb, in_=bass.AP(
            tensor=gate_d.ap().tensor, offset=0, ap=[[0, P], [1, 1]]))

        def load_w(d, n):
            t = const.tile([P, KT, n], BF16, tag=f"w_{d.name}", name=f"w_{d.name}")
            sync.dma_start(out=t, in_=d.ap().rearrange("(k p) n -> p k n", p=P))
            return t

        wqkv_sb = load_w(wqkv_d, 3 * HL)
        wsa_sb = load_w(wsa_d, D)
        wcq_sb = load_w(wcq_d, HL)
        wckv_sb = load_w(wckv_d, 2 * HL)
        wca_sb = load_w(wca_d, D)
        w1_sb = load_w(w1_d, HIDL)
        w2_sb = const.tile([P, HIDL // P, D], BF16)
        sync.dma_start(out=w2_sb, in_=w2_d.ap().rearrange("(k p) n -> p k n", p=P))

        encT_sb = const.tile([P, KT, S], BF16)
        sync.dma_start(out=encT_sb, in_=encT_d.ap().rearrange("(k p) t -> p k t", p=P))

        x_sb = xres.tile([P, NB, D], F32)
        sync.dma_start(out=x_sb, in_=x_d.ap().rearrange("(n p) d -> p n d", p=P))

        gb = {}
        for nm, gd, bd, ap_f in (("ln1", g1_d, b1ln_d, apply_gb1),
                                 ("ln2", g2_d, b2ln_d, apply_gb2),
                                 ("ln3", g3_d, b3ln_d, apply_gb3)):
            if ap_f:
                gb[nm] = (bcast_row(gd.ap(), D), bcast_row(bd.ap(), D))
        qb_sb = col_vec(qb_d.ap(), HL) if qb_nz else None
        kb_sb = col_vec(kb_d.ap(), HL) if kb_nz else None
        cqb_sb = col_vec(cqb_d.ap(), HL) if cqb_nz else None
        ckb_sb = col_vec(ckb_d.ap(), HL) if ckb_nz else None
        vb_sb = bcast_row(vb_d.ap(), HL) if vb_nz else None
        cvb_sb = bcast_row(cvb_d.ap(), HL) if cvb_nz else None
        sab_sb = bcast_row(sab_d.ap(), D) if sab_nz else None
        cab_sb = bcast_row(cab_d.ap(), D) if cab_nz else None
        mb1_sb = col_vec(mb1_d.ap(), HIDL) if b1_nz else None  # [512,1] -> use [:, hch]
        mb1_cols = None
        if b1_nz:
            mb1_cols = const.tile([P, HIDL // P], F32)
            sync.dma_start(out=mb1_cols,
                           in_=mb1_d.ap().rearrange("(k p) -> p k", p=P))
        mb2_sb = bcast_row(mb2_d.ap(), D) if b2_nz else None

        tap_ctr = [0]

        def tap(name, ap):
            if not DEBUG_TAPS:
                return
            d = nc.dram_tensor(f"dbg_{name}", list(ap.shape), ap.dtype,
                               kind="ExternalOutput")
            sync.dma_start(out=d.ap(), in_=ap)

        # ---------------- helpers -----------------
        def layernorm_transpose(ln_name, out_xT, n_blocks=NB):
            """LN(x_sb) (token-major stats) -> bf16 -> transpose into out_xT [P, KT, T]."""
            gbp = gb.get(ln_name)
            for blk in range(n_blocks):
                stats = work.tile([P, 6], F32, tag="stats")
                nc.vector.bn_stats(out=stats, in_=x_sb[:, blk])
                mv = work.tile([P, 2], F32, tag="mv")
                nc.vector.bn_aggr(out=mv, in_=stats)
                # rstd = sqrt(1/(var+eps)); DVE approx reciprocal avoids the
                # walrus sync-wait limit on InstReciprocal.
                rr = work.tile([P, 1], F32, tag="rr")
                nc.vector.tensor_scalar(out=rr, in0=mv[:, 1:2], scalar1=float(EPS),
                                        scalar2=None, op0=mybir.AluOpType.add)
                nc.vector.reciprocal_approx_fast(out=mv[:, 1:2], in_=rr)
                nc.scalar.activation(out=mv[:, 1:2], in_=mv[:, 1:2],
                                     func=mybir.ActivationFunctionType.Sqrt,
                                     scale=1.0)
                xn = work.tile([P, D], BF16, tag="xnorm")
                if gbp is None:
                    nc.vector.tensor_scalar(out=xn, in0=x_sb[:, blk],
                                            scalar1=mv[:, 0:1], scalar2=mv[:, 1:2],
                                            op0=mybir.AluOpType.subtract,
                                            op1=mybir.AluOpType.mult)
                else:
                    xf = work.tile([P, D], F32, tag="xnorm_f")
                    nc.vector.tensor_scalar(out=xf, in0=x_sb[:, blk],
                                            scalar1=mv[:, 0:1], scalar2=mv[:, 1:2],
                                            op0=mybir.AluOpType.subtract,
                                            op1=mybir.AluOpType.mult)
                    nc.vector.tensor_mul(out=xf, in0=xf, in1=gbp[0])
                    nc.vector.tensor_add(out=xn, in0=xf, in1=gbp[1])
                pst = psG.tile([P, D], BF16, tag="ps")
                for kt in range(KT):
                    nc.tensor.transpose(pst[:, kt * P:(kt + 1) * P],
                                        xn[:, kt * P:(kt + 1) * P], ident)
                nc.vector.tensor_copy(
                    out=out_xT[:, :, blk * P:(blk + 1) * P],
                    in_=pst.rearrange("p (k t) -> p k t", k=KT))

        def mm_TN(out_sb, w_sb, w_col0, w_cols, rhs_T, bias_col=None):
            """out_sb[M=w_cols rows, T] (bf16) = w[:, w_col0:+w_cols].T @ rhs_T.
            Contracts over D (KT tiles); 1024-wide PSUM tiles, single copy out."""
            for nch in range(T // 1024):
                ps = psG.tile([P, 1024], F32, tag="ps")
                for half in range(2):
                    for kt in range(KT):
                        nc.tensor.matmul(
                            ps[:w_cols, half * 512:(half + 1) * 512],
                            w_sb[:, kt, w_col0:w_col0 + w_cols],
                            rhs_T[:, kt, nch * 1024 + half * 512:nch * 1024 + (half + 1) * 512],
                            start=(kt == 0), stop=(kt == KT - 1))
                if bias_col is None:
                    nc.vector.tensor_copy(out=out_sb[:w_cols, nch * 1024:(nch + 1) * 1024],
                                          in_=ps[:w_cols])
                else:
                    nc.vector.tensor_scalar(out=out_sb[:w_cols, nch * 1024:(nch + 1) * 1024],
                                            in0=ps[:w_cols], scalar1=bias_col,
                                            scalar2=None, op0=mybir.AluOpType.add)

        def mm_val(v_sb, src_T, w_sb, w_col0, bias_b=None):
            """v_sb [P, NB, 130] token-major values (+ones cols) = src.T @ w[:, w_col0:+128]."""
            nc.vector.memset(
                v_sb.rearrange("p n (two c) -> p n two c", two=2)[:, :, :, HD:HD + 1], 1.0)
            for blk in range(NB):
                ps = psG.tile([P, HL], F32, tag="ps")
                for kt in range(KT):
                    nc.tensor.matmul(ps, src_T[:, kt, blk * P:(blk + 1) * P],
                                     w_sb[:, kt, w_col0:w_col0 + HL],
                                     start=(kt == 0), stop=(kt == KT - 1))
                dst = v_sb[:, blk].rearrange("p (two c) -> p two c", two=2)[:, :, :HD]
                src = ps.rearrange("p (two c) -> p two c", two=2)
                if bias_b is None:
                    nc.vector.tensor_copy(out=dst, in_=src)
                else:
                    bb = bias_b.rearrange("p (two c) -> p two c", two=2)
                    nc.vector.tensor_add(out=dst, in0=src, in1=bb)

        def attention(qT, kT, v_sb, attnT_loc, mode, maskT_d, n_kb=NB):
            """attnT_loc [P(2 heads*64), T] bf16 = softmax(qk^T/8, mask) @ v, transposed.
            mode: 'causal' | 'ones' | 'general'. Processes query chunks in pairs
            (1024 queries) so exp runs as one wide ACT instruction."""
            for h in range(2):
                qh = qT[h * HD:(h + 1) * HD]
                kh = kT[h * HD:(h + 1) * HD]
                for qcp in range(NCH // 2):
                    q0 = 2 * qcp           # first 512-chunk of this pair
                    col0 = q0 * 512
                    # keys needed: causal -> kb <= 4*(q0+1)+3
                    kbs = range(min(n_kb, 4 * (q0 + 1) + 4) if mode == "causal" else n_kb)
                    avT = psB.tile([P, 2, 512], F32, tag="avT")
                    for kb in kbs:
                        if mode == "causal":
                            qcs = [qc for qc in (q0, q0 + 1) if 4 * qc + 3 >= kb]
                        else:
                            qcs = [q0, q0 + 1]
                        w = 512 * len(qcs)
                        lo = (qcs[0] - q0) * 512
                        pr = probs_p.tile([P, 1024], BF16, tag="probs")
                        if mode == "general":
                            m_sb = probs_p.tile([P, 1024], BF16, tag="mask")
                            sync.dma_start(
                                out=m_sb,
                                in_=maskT_d.ap()[kb * P:(kb + 1) * P, col0:col0 + 1024])
                        ps = psG.tile([P, 1024], F32, tag="ps")
                        for qc in qcs:
                            nc.tensor.matmul(ps[:, (qc - q0) * 512:(qc - q0 + 1) * 512],
                                             kh[:, kb * P:(kb + 1) * P],
                                             qh[:, qc * 512:(qc + 1) * 512],
                                             start=True, stop=True)
                        nc.scalar.activation(out=pr[:, lo:lo + w], in_=ps[:, lo:lo + w],
                                             func=mybir.ActivationFunctionType.Exp,
                                             scale=float(HD) ** -0.5)
                        if mode == "causal" and kb // 4 in qcs:
                            qc = kb // 4
                            j = kb % 4
                            # keep prob where key k <= query q: (q - k - 128j) >= 0
                            nc.gpsimd.affine_select(
                                out=pr[:, (qc - q0) * 512:(qc - q0 + 1) * 512],
                                in_=pr[:, (qc - q0) * 512:(qc - q0 + 1) * 512],
                                pattern=[[1, 512]], channel_multiplier=-1,
                                base=-128 * j, compare_op=mybir.AluOpType.is_ge,
                                fill=0.0)
                        elif mode == "general":
                            nc.vector.tensor_mul(out=pr[:, lo:lo + w],
                                                 in0=pr[:, lo:lo + w],
                                                 in1=m_sb[:, lo:lo + w])
                        for qc in qcs:
                            last_kb = (min(n_kb - 1, 4 * qc + 3)
                                       if mode == "causal" else n_kb - 1)
                            nc.tensor.matmul(
                                avT[:HD + 1, qc - q0],
                                v_sb[:, kb, h * (HD + 1):(h + 1) * (HD + 1)],
                                pr[:, (qc - q0) * 512:(qc - q0 + 1) * 512],
                                start=(kb == 0), stop=(kb == last_kb))
                    rec = work.tile([HD + 1, 1024], F32, tag="rec", bufs=2)
                    nc.vector.tensor_copy(out=rec[HD:HD + 1],
                                          in_=avT[HD:HD + 1].rearrange("p a b -> p (a b)"))
                    # broadcast the sums row to partitions 0..63 via a DRAM
                    # bounce (step-0 partition APs are only legal on DRAM).
                    rec_d = dram.tile([1, 1024], F32, tag="rec_d", bufs=2)
                    nc.gpsimd.dma_start(out=rec_d, in_=rec[HD:HD + 1])
                    src = bass.AP(tensor=rec_d.tensor, offset=rec_d.offset,
                                  ap=[[0, HD]] + list(rec_d.ap)[1:])
                    nc.gpsimd.dma_start(out=rec[:HD], in_=src)
                    rec2 = work.tile([HD, 1024], F32, tag="rec2", bufs=2)
                    nc.vector.reciprocal(out=rec2, in_=rec[:HD])
                    if DEBUG_TAPS and h == 0:
                        tap_ctr[0] += 1
                        tap(f"sums{tap_ctr[0]}", rec[HD:HD + 1])
                        tap(f"rec{tap_ctr[0]}", rec2)
                    nc.vector.tensor_mul(
                        out=attnT_loc[h * HD:(h + 1) * HD,
                                      col0:col0 + 1024].rearrange("p (a b) -> p a b", a=2),
                        in0=avT[:HD],
                        in1=rec2.rearrange("p (a b) -> p a b", a=2))

        def proj_residual(attnT_full, w_sb, bias_row):
            """x_sb += attnT_full.T @ w (+bias)."""
            for blk in range(NB):
                ps = psG.tile([P, D], F32, tag="ps")
                for kt in range(KT):
                    nc.tensor.matmul(ps, attnT_full[:, kt, blk * P:(blk + 1) * P],
                                     w_sb[:, kt, :], start=(kt == 0), stop=(kt == KT - 1))
                nc.vector.tensor_add(out=x_sb[:, blk], in0=x_sb[:, blk], in1=ps)
                if bias_row is not None:
                    nc.vector.tensor_add(out=x_sb[:, blk], in0=x_sb[:, blk], in1=bias_row)

        # ================ pipeline ================
        xT = big.tile([P, KT, T], BF16, tag="xT", name="x1T")
        layernorm_transpose("ln1", xT)
        tap("x1T", xT)

        qT = big.tile([P, T], BF16, tag="qT", name="qT_self")
        kT = big.tile([P, T], BF16, tag="kT", name="kT_self")
        v_sb = big.tile([P, NB, 2 * (HD + 1)], BF16, tag="v", name="v_self")
        mm_TN(qT, wqkv_sb, 0, HL, xT, qb_sb)
        mm_TN(kT, wqkv_sb, HL, HL, xT, kb_sb)
        mm_val(v_sb, xT, wqkv_sb, 2 * HL, vb_sb)
        tap("qT", qT)
        tap("kT", kT)
        tap("v", v_sb)

        attnT_loc = big.tile([P, T], BF16, tag="attnT", name="attnT_sa")
        attention(qT, kT, v_sb, attnT_loc, self_mode, maskT_self_d)
        tap("attnT_sa", attnT_loc)

        # cross-attention K/V depend only on the encoder; emit them here so the
        # scheduler overlaps their matmuls with the AllGather + projection.
        kcT = big.tile([P, T], BF16, tag="kcT", name="kT_cross")
        vc_sb = big.tile([P, NB, 2 * (HD + 1)], BF16, tag="vc", name="v_cross")
        mm_TN(kcT, wckv_sb, 0, HL, encT_sb, ckb_sb)
        mm_val(vc_sb, encT_sb, wckv_sb, HL, cvb_sb)

        # AllGather self-attention heads (bf16)
        ag1_in = dram.tile([P, T], BF16, name="ag1_in")
        ag1_out = dram.tile([G * P, T], BF16, name="ag1_out")
        nc.gpsimd.dma_start(out=ag1_in, in_=attnT_loc)
        if sim1:
            for r in range(G):
                nc.gpsimd.dma_start(out=ag1_out[r * P:(r + 1) * P, :], in_=ag1_in)
        else:
            nc.gpsimd.collective_compute(
                "AllGather", mybir.AluOpType.bypass,
                replica_groups=[[0, 1, 2, 3], [4, 5, 6, 7]],
                ins=[ag1_in.opt()], outs=[ag1_out.opt()])
        attnT_full = big.tile([P, KT, T], BF16, tag="attnT_full", name="attnT_sa_full")
        sync.dma_start(out=attnT_full,
                       in_=ag1_out.rearrange("(k p) t -> p k t", p=P))
        proj_residual(attnT_full, wsa_sb, sab_sb)
        tap("x_after_sa", x_sb)

        # ---- cross attention ----
        layernorm_transpose("ln2", xT)  # xT now holds x2T
        qcT = big.tile([P, T], BF16, tag="qcT", name="qT_cross")
        mm_TN(qcT, wcq_sb, 0, HL, xT, cqb_sb)
        attnT_ca = big.tile([P, T], BF16, tag="attnT_ca", name="attnT_ca")
        attention(qcT, kcT, vc_sb, attnT_ca, cross_mode, maskT_cross_d, n_kb=S // P)

        ag2_in = dram.tile([P, T], BF16, name="ag2_in")
        ag2_out = dram.tile([G * P, T], BF16, name="ag2_out")
        nc.gpsimd.dma_start(out=ag2_in, in_=attnT_ca)
        if sim1:
            for r in range(G):
                nc.gpsimd.dma_start(out=ag2_out[r * P:(r + 1) * P, :], in_=ag2_in)
        else:
            nc.gpsimd.collective_compute(
                "AllGather", mybir.AluOpType.bypass,
                replica_groups=[[0, 1, 2, 3], [4, 5, 6, 7]],
                ins=[ag2_in.opt()], outs=[ag2_out.opt()])
        sync.dma_start(out=attnT_full,
                       in_=ag2_out.rearrange("(k p) t -> p k t", p=P))
        proj_residual(attnT_full, wca_sb, cab_sb)
        tap("x_after_ca", x_sb)

        # ---- MLP (hidden-slice tensor parallel) ----
        layernorm_transpose("ln3", xT)  # xT now holds x3T
        hT = big.tile([P, HIDL // P, T], BF16, tag="hT", name="hT")
        for hch in range(HIDL // P):
            for nch2 in range(T // 1024):
                ps = psG.tile([P, 1024], F32, tag="ps")
                for half in range(2):
                    c0 = nch2 * 1024 + half * 512
                    for kt in range(KT):
                        nc.tensor.matmul(ps[:, half * 512:(half + 1) * 512],
                                         w1_sb[:, kt, hch * P:(hch + 1) * P],
                                         xT[:, kt, c0:c0 + 512],
                                         start=(kt == 0), stop=(kt == KT - 1))
                nc.scalar.activation(
                    out=hT[:, hch, nch2 * 1024:(nch2 + 1) * 1024], in_=ps,
                    func=mybir.ActivationFunctionType.Gelu,
                    bias=(mb1_cols[:, hch:hch + 1] if b1_nz else 0.0), scale=1.0)

        rs_in = dram.tile([T, D], F32, name="rs_in")
        rs_out = dram.tile([T // G, D], F32, name="rs_out")
        for blk in range(NB):
            ps = psG.tile([P, D], F32, tag="ps")
            for hch in range(HIDL // P):
                nc.tensor.matmul(ps, hT[:, hch, blk * P:(blk + 1) * P],
                                 w2_sb[:, hch, :], start=(hch == 0),
                                 stop=(hch == HIDL // P - 1))
            part = work.tile([P, D], F32, tag="part")
            nc.vector.scalar_tensor_tensor(out=part, in0=x_sb[:, blk], scalar=gate_sb[:, 0:1],
                                           in1=ps, op0=mybir.AluOpType.mult,
                                           op1=mybir.AluOpType.add)
            if b2_nz:
                nc.vector.tensor_add(out=part, in0=part, in1=mb2_sb)
            sync.dma_start(out=rs_in[blk * P:(blk + 1) * P, :], in_=part)

        if sim1:
            nc.gpsimd.dma_start(out=rs_out, in_=rs_in[:T // G, :])
        else:
            nc.gpsimd.collective_compute(
                "ReduceScatter", mybir.AluOpType.add,
                replica_groups=[[0, 1, 2, 3], [4, 5, 6, 7]],
                ins=[rs_in.opt()], outs=[rs_out.opt()])
        sync.dma_start(out=out_d.ap(), in_=rs_out)

        for p in reversed((const, xres, big, work, probs_p, dram, psG, psB)):
            p.release()

    nc.compile()
    return nc


def prepare(inputs):
    """Host-side prep: specialization flags, program build, per-core input maps.
    Returns (nc, in_maps)."""
    x = np.asarray(inputs["x"], np.float32)
    enc = np.asarray(inputs["encoder_out"], np.float32)
    self_mask = np.asarray(inputs["self_mask"]).astype(bool)[0, 0]
    cross_mask = np.asarray(inputs["cross_mask"]).astype(bool)[0, 0]
    qkv_w = np.asarray(inputs["qkv_w"], np.float32)
    qkv_b = np.asarray(inputs["qkv_b"], np.float32)
    sa_proj_w = np.asarray(inputs["sa_proj_w"], np.float32)
    sa_proj_b = np.asarray(inputs["sa_proj_b"], np.float32)
    ln1_g = np.asarray(inputs["ln1_g"], np.float32)
    ln1_b = np.asarray(inputs["ln1_b"], np.float32)
    q_w = np.asarray(inputs["q_w"], np.float32)
    q_b = np.asarray(inputs["q_b"], np.float32)
    kv_w = np.asarray(inputs["kv_w"], np.float32)
    kv_b = np.asarray(inputs["kv_b"], np.float32)
    ca_proj_w = np.asarray(inputs["ca_proj_w"], np.float32)
    ca_proj_b = np.asarray(inputs["ca_proj_b"], np.float32)
    ln2_g = np.asarray(inputs["ln2_g"], np.float32)
    ln2_b = np.asarray(inputs["ln2_b"], np.float32)
    mlp_w1 = np.asarray(inputs["mlp_w1"], np.float32)
    mlp_b1 = np.asarray(inputs["mlp_b1"], np.float32)
    mlp_w2 = np.asarray(inputs["mlp_w2"], np.float32)
    mlp_b2 = np.asarray(inputs["mlp_b2"], np.float32)
    ln3_g = np.asarray(inputs["ln3_g"], np.float32)
    ln3_b = np.asarray(inputs["ln3_b"], np.float32)

    def mask_mode(m):
        if m.all():
            return "ones"
        if np.array_equal(m, np.tril(np.ones(m.shape, bool))):
            return "causal"
        return "general"

    self_mode = mask_mode(self_mask)
    cross_mode = mask_mode(cross_mask)
    if cross_mode == "causal":  # causal path only wired for the self block
        cross_mode = "general"

    def nz(a):
        return bool(np.any(a != 0.0))

    def nontriv(g, b):
        return bool(np.any(g != 1.0) or np.any(b != 0.0))

    cfg = (self_mode, cross_mode,
           nontriv(ln1_g, ln1_b), nontriv(ln2_g, ln2_b), nontriv(ln3_g, ln3_b),
           nz(qkv_b[:D]), nz(qkv_b[D:2 * D]), nz(qkv_b[2 * D:]),
           nz(sa_proj_b), nz(q_b), nz(kv_b[:D]), nz(kv_b[D:]), nz(ca_proj_b),
           nz(mlp_b1), nz(mlp_b2))

    global _last_cfg
    _last_cfg = cfg
    if cfg not in _cache:
        _cache[cfg] = _build(cfg)
    nc = _cache[cfg]

    bf = lambda a: np.ascontiguousarray(a.astype(BF16_NP))
    in_maps = []
    for core in range(8):
        g, r = divmod(core, G)
        hc = slice(r * HL, (r + 1) * HL)       # this core's 128 head columns
        hid = slice(r * HIDL, (r + 1) * HIDL)  # this core's mlp hidden slice
        m = {
            "x": np.ascontiguousarray(x[g]),
            "encT": bf(enc[g].T),
            "wqkv": bf(np.concatenate(
                [qkv_w[:, hc], qkv_w[:, D:][:, hc], qkv_w[:, 2 * D:][:, hc]], axis=1)),
            "wsa": bf(sa_proj_w),
            "wcq": bf(q_w[:, hc]),
            "wckv": bf(np.concatenate([kv_w[:, :D][:, hc], kv_w[:, D:][:, hc]], axis=1)),
            "wca": bf(ca_proj_w),
            "w1": bf(mlp_w1[:, hid]),
            "w2": bf(mlp_w2[hid, :]),
            "gate": np.full((1, 1), 1.0 if r == 0 else 0.0, np.float32),
        }
        if self_mode == "general":
            m["maskT_self"] = bf(self_mask.T.astype(np.float32))
        if cross_mode == "general":
            m["maskT_cross"] = bf(cross_mask.T.astype(np.float32))
        if cfg[2]:
            m["ln1_g"], m["ln1_b"] = ln1_g, ln1_b
        if cfg[3]:
            m["ln2_g"], m["ln2_b"] = ln2_g, ln2_b
        if cfg[4]:
            m["ln3_g"], m["ln3_b"] = ln3_g, ln3_b
        if cfg[5]:
            m["q_bias"] = np.ascontiguousarray(qkv_b[:D][hc])
        if cfg[6]:
            m["k_bias"] = np.ascontiguousarray(qkv_b[D:2 * D][hc])
        if cfg[7]:
            m["v_bias"] = np.ascontiguousarray(qkv_b[2 * D:][hc])
        if cfg[8]:
            m["sa_bias"] = sa_proj_b
        if cfg[9]:
            m["cq_bias"] = np.ascontiguousarray(q_b[hc])
        if cfg[10]:
            m["ck_bias"] = np.ascontiguousarray(kv_b[:D][hc])
        if cfg[11]:
            m["cv_bias"] = np.ascontiguousarray(kv_b[D:][hc])
        if cfg[12]:
            m["ca_bias"] = ca_proj_b
        if cfg[13]:
            m["mlp_b1"] = np.ascontiguousarray(mlp_b1[hid])
        if cfg[14]:
            m["mlp_b2_gated"] = mlp_b2 if r == 0 else np.zeros_like(mlp_b2)
        in_maps.append(m)

    return nc, in_maps


def assemble(results) -> np.ndarray:
    out = np.empty((B, T, D), np.float32)
    for core in range(8):
        g, r = divmod(core, G)
        out[g, r * (T // G):(r + 1) * (T // G), :] = results[core]["out"]
    return out


def kernel(**inputs) -> np.ndarray:
    nc, in_maps = prepare(inputs)
    res = run_bass_kernel_spmd(nc, in_maps, core_ids=list(range(8)))
    global LAST_RESULTS
    LAST_RESULTS = res
    return assemble(res.results)


# revision 44
# speedup vs baseline: 265.1585x; 1.0050x over previous
"""Trainium2 Bass kernel for a transformer decoder block (self-attn + cross-attn + MLP).

Sharding (8 NeuronCores):
  - 2 groups of 4 cores; group g handles batch b=g (data parallel on B=2).
  - Within a group, rank r in {0..3} owns head pair (2r, 2r+1) for both attention
    blocks (tensor parallel on H=8) and MLP hidden slice [512r:512(r+1)]
    (tensor parallel on MLP_H=2048).
  - LayerNorms are computed replicated (full T) on every core.
  - Attention head outputs are exchanged with a small bf16 AllGather (512KB/rank);
    each core then runs the output projection itself (cheap, full T).
  - The MLP second matmul produces partial sums; the residual stream is folded in
    on rank 0 of each group, and one ReduceScatter(add) both sums the partials and
    token-shards the final output: core (g, r) emits output tokens [512r:512(r+1)]
    of batch g. The host reassembles the full [B, T, D] output.

All matmuls run in bf16 (fp32 accumulation); LayerNorm statistics, softmax
normalization and the residual stream stay in fp32. Softmax skips the max
subtraction (scores are O(1) for these scales) and applies masks multiplicatively
after exp. Host-side specialization (legal: the program is compiled per call):
  - tril self-mask -> causal block skipping + on-chip affine triangle masks
  - all-ones masks -> no masking
  - all-zero biases / unit LayerNorm gains -> skipped
  - otherwise a general path applies masks / biases / gains from extra inputs.
"""

import numpy as np
import ml_dtypes

import concourse.bass as bass
import concourse.mybir as mybir
import concourse.tile as tile
from concourse import bacc
from concourse.bass_utils import run_bass_kernel_spmd
from concourse.masks import make_identity

B, T, S, D, H = 2, 2048, 2048, 512, 8
HD = D // H          # 64
MLP_H = 4 * D        # 2048
EPS = 1e-5
P = 128
NB = T // P          # 16 token blocks
KT = D // P          # 4 contraction tiles over D
NCH = T // 512       # 4 query/token 512-chunks
G = 4                # cores per group
HL = 2 * HD          # 128 local head columns (2 heads)
HIDL = MLP_H // G    # 512 local mlp hidden
F32 = mybir.dt.float32
BF16 = mybir.dt.bfloat16
BF16_NP = ml_dtypes.bfloat16

_cache = {}
DEBUG_TAPS = False  # when True, _build adds intermediate tensors as extra outputs
LAST_RESULTS = None


def _build(cfg, sim1=False):
    """Build the (SPMD-identical) Bass program for one core. cfg is a tuple of
    specialization flags. sim1=True replaces collectives with equivalent-byte
    local DMA copies so TimelineSim (single-core, no collectives) can run."""
    (self_mode, cross_mode, apply_gb1, apply_gb2, apply_gb3,
     qb_nz, kb_nz, vb_nz, sab_nz, cqb_nz, ckb_nz, cvb_nz, cab_nz,
     b1_nz, b2_nz) = cfg

    nc = bacc.Bacc("TRN2", debug=False, num_devices=8)

    # ---------------- I/O -----------------
    x_d = nc.dram_tensor("x", [T, D], F32, kind="ExternalInput")
    encT_d = nc.dram_tensor("encT", [D, S], BF16, kind="ExternalInput")
    wqkv_d = nc.dram_tensor("wqkv", [D, 3 * HL], BF16, kind="ExternalInput")
    wsa_d = nc.dram_tensor("wsa", [D, D], BF16, kind="ExternalInput")
    wcq_d = nc.dram_tensor("wcq", [D, HL], BF16, kind="ExternalInput")
    wckv_d = nc.dram_tensor("wckv", [D, 2 * HL], BF16, kind="ExternalInput")
    wca_d = nc.dram_tensor("wca", [D, D], BF16, kind="ExternalInput")
    w1_d = nc.dram_tensor("w1", [D, HIDL], BF16, kind="ExternalInput")
    w2_d = nc.dram_tensor("w2", [HIDL, D], BF16, kind="ExternalInput")
    gate_d = nc.dram_tensor("gate", [1, 1], F32, kind="ExternalInput")
    out_d = nc.dram_tensor("out", [T // G, D], F32, kind="ExternalOutput")

    maskT_self_d = maskT_cross_d = None
    if self_mode == "general":
        maskT_self_d = nc.dram_tensor("maskT_self", [T, T], BF16, kind="ExternalInput")
    if cross_mode == "general":
        maskT_cross_d = nc.dram_tensor("maskT_cross", [S, T], BF16, kind="ExternalInput")

    # general-path params (broadcast rows in DRAM -> [128, N] SBUF via step-0 AP)
    def opt_in(name, shape, dt=F32, cond=True):
        return nc.dram_tensor(name, shape, dt, kind="ExternalInput") if cond else None

    g1_d = opt_in("ln1_g", [D], cond=apply_gb1)
    b1ln_d = opt_in("ln1_b", [D], cond=apply_gb1)
    g2_d = opt_in("ln2_g", [D], cond=apply_gb2)
    b2ln_d = opt_in("ln2_b", [D], cond=apply_gb2)
    g3_d = opt_in("ln3_g", [D], cond=apply_gb3)
    b3ln_d = opt_in("ln3_b", [D], cond=apply_gb3)
    qb_d = opt_in("q_bias", [HL], cond=qb_nz)      # per-partition column
    kb_d = opt_in("k_bias", [HL], cond=kb_nz)
    vb_d = opt_in("v_bias", [HL], cond=vb_nz)      # per-free broadcast
    sab_d = opt_in("sa_bias", [D], cond=sab_nz)
    cqb_d = opt_in("cq_bias", [HL], cond=cqb_nz)
    ckb_d = opt_in("ck_bias", [HL], cond=ckb_nz)
    cvb_d = opt_in("cv_bias", [HL], cond=cvb_nz)
    cab_d = opt_in("ca_bias", [D], cond=cab_nz)
    mb1_d = opt_in("mlp_b1", [HIDL], cond=b1_nz)
    mb2_d = opt_in("mlp_b2_gated", [D], cond=b2_nz)

    with tile.TileContext(nc) as tc:
        const = tc.alloc_tile_pool(name="const", bufs=1)
        xres = tc.alloc_tile_pool(name="xres", bufs=1)
        big = tc.alloc_tile_pool(name="big", bufs=1)
        work = tc.alloc_tile_pool(name="work", bufs=3)
        probs_p = tc.alloc_tile_pool(name="probs", bufs=4)
        dram = tc.alloc_tile_pool(name="dram", bufs=1, space="DRAM")
        # PSUM budget (8 banks): "ps" [128,1024]f32 x2 bufs = 4 banks shared by
        # scores and all general matmul outputs; "avT" [65,2,512] x2 bufs = 4.
        psG = tc.alloc_tile_pool(name="psG", bufs=2, space="PSUM")
        psB = tc.alloc_tile_pool(name="psB", bufs=2, space="PSUM")

        sync = nc.sync

        def bcast_row(dram_ap, n, dt=F32, parts=P):
            """[n] DRAM -> [parts, n] SBUF, replicated across partitions."""
            t = const.tile([parts, n], dt, tag=f"bc_{dram_ap.tensor.name}",
                           name=f"bc_{dram_ap.tensor.name}")
            src = bass.AP(tensor=dram_ap.tensor, offset=dram_ap.offset,
                          ap=[[0, parts]] + list(dram_ap.ap))
            nc.gpsimd.dma_start(out=t, in_=src)
            return t

        def col_vec(dram_ap, n):
            """[n] DRAM -> [n, 1] SBUF column (per-partition scalar)."""
            t = const.tile([n, 1], F32, tag=f"cv_{dram_ap.tensor.name}",
                           name=f"cv_{dram_ap.tensor.name}")
            sync.dma_start(out=t, in_=dram_ap.rearrange("n -> n 1"))
            return t

        # ---------------- constants / weights -----------------
        ident = const.tile([P, P], BF16)
        make_identity(nc, ident)
        eps_sb = const.tile([P, 1], F32)
        nc.vector.memset(eps_sb, EPS)
        gate_sb = const.tile([P, 1], F32)
        nc.gpsimd.dma_start(out=gate_sb, in_=bass.AP(
            tensor=gate_d.ap().tensor, offset=0, ap=[[0, P], [1, 1]]))

        def load_w(d, n):
            t = const.tile([P, KT, n], BF16, tag=f"w_{d.name}", name=f"w_{d.name}")
            sync.dma_start(out=t, in_=d.ap().rearrange("(k p) n -> p k n", p=P))
            return t

        wqkv_sb = load_w(wqkv_d, 3 * HL)
        wsa_sb = load_w(wsa_d, D)
        wcq_sb = load_w(wcq_d, HL)
        wckv_sb = load_w(wckv_d, 2 * HL)
        wca_sb = load_w(wca_d, D)
        w1_sb = load_w(w1_d, HIDL)
        w2_sb = const.tile([P, HIDL // P, D], BF16)
        sync.dma_start(out=w2_sb, in_=w2_d.ap().rearrange("(k p) n -> p k n", p=P))

        encT_sb = const.tile([P, KT, S], BF16)
        sync.dma_start(out=encT_sb, in_=encT_d.ap().rearrange("(k p) t -> p k t", p=P))

        x_sb = xres.tile([P, NB, D], F32)
        sync.dma_start(out=x_sb, in_=x_d.ap().rearrange("(n p) d -> p n d", p=P))

        gb = {}
        for nm, gd, bd, ap_f in (("ln1", g1_d, b1ln_d, apply_gb1),
                                 ("ln2", g2_d, b2ln_d, apply_gb2),
                                 ("ln3", g3_d, b3ln_d, apply_gb3)):
            if ap_f:
                gb[nm] = (bcast_row(gd.ap(), D), bcast_row(bd.ap(), D))
        qb_sb = col_vec(qb_d.ap(), HL) if qb_nz else None
        kb_sb = col_vec(kb_d.ap(), HL) if kb_nz else None
        cqb_sb = col_vec(cqb_d.ap(), HL) if cqb_nz else None
        ckb_sb = col_vec(ckb_d.ap(), HL) if ckb_nz else None
        vb_sb = bcast_row(vb_d.ap(), HL) if vb_nz else None
        cvb_sb = bcast_row(cvb_d.ap(), HL) if cvb_nz else None
        sab_sb = bcast_row(sab_d.ap(), D) if sab_nz else None
        cab_sb = bcast_row(cab_d.ap(), D) if cab_nz else None
        mb1_sb = col_vec(mb1_d.ap(), HIDL) if b1_nz else None  # [512,1] -> use [:, hch]
        mb1_cols = None
        if b1_nz:
            mb1_cols = const.tile([P, HIDL // P], F32)
            sync.dma_start(out=mb1_cols,
                           in_=mb1_d.ap().rearrange("(k p) -> p k", p=P))
        mb2_sb = bcast_row(mb2_d.ap(), D) if b2_nz else None

        tap_ctr = [0]

        def tap(name, ap):
            if not DEBUG_TAPS:
                return
            d = nc.dram_tensor(f"dbg_{name}", list(ap.shape), ap.dtype,
                               kind="ExternalOutput")
            sync.dma_start(out=d.ap(), in_=ap)

        # ---------------- helpers -----------------
        def layernorm_transpose(ln_name, out_xT, n_blocks=NB):
            """LN(x_sb) (token-major stats) -> bf16 -> transpose into out_xT [P, KT, T]."""
            gbp = gb.get(ln_name)
            for blk in range(n_blocks):
                stats = work.tile([P, 6], F32, tag="stats")
                nc.vector.bn_stats(out=stats, in_=x_sb[:, blk])
                mv = work.tile([P, 2], F32, tag="mv")
                nc.vector.bn_aggr(out=mv, in_=stats)
                # rstd = sqrt(1/(var+eps)); DVE approx reciprocal avoids the
                # walrus sync-wait limit on InstReciprocal.
                rr = work.tile([P, 1], F32, tag="rr")
                nc.vector.tensor_scalar(out=rr, in0=mv[:, 1:2], scalar1=float(EPS),
                                        scalar2=None, op0=mybir.AluOpType.add)
                nc.vector.reciprocal_approx_fast(out=mv[:, 1:2], in_=rr)
                nc.scalar.activation(out=mv[:, 1:2], in_=mv[:, 1:2],
                                     func=mybir.ActivationFunctionType.Sqrt,
                                     scale=1.0)
                xn = work.tile([P, D], BF16, tag="xnorm")
                if gbp is None:
                    nc.vector.tensor_scalar(out=xn, in0=x_sb[:, blk],
                                            scalar1=mv[:, 0:1], scalar2=mv[:, 1:2],
                                            op0=mybir.AluOpType.subtract,
                                            op1=mybir.AluOpType.mult)
                else:
                    xf = work.tile([P, D], F32, tag="xnorm_f")
                    nc.vector.tensor_scalar(out=xf, in0=x_sb[:, blk],
                                            scalar1=mv[:, 0:1], scalar2=mv[:, 1:2],
                                            op0=mybir.AluOpType.subtract,
                                            op1=mybir.AluOpType.mult)
                    nc.vector.tensor_mul(out=xf, in0=xf, in1=gbp[0])
                    nc.vector.tensor_add(out=xn, in0=xf, in1=gbp[1])
                pst = psG.tile([P, D], BF16, tag="ps")
                for kt in range(KT):
                    nc.tensor.transpose(pst[:, kt * P:(kt + 1) * P],
                                        xn[:, kt * P:(kt + 1) * P], ident)
                nc.vector.tensor_copy(
                    out=out_xT[:, :, blk * P:(blk + 1) * P],
                    in_=pst.rearrange("p (k t) -> p k t", k=KT))

        def mm_TN(out_sb, w_sb, w_col0, w_cols, rhs_T, bias_col=None):
            """out_sb[M=w_cols rows, T] (bf16) = w[:, w_col0:+w_cols].T @ rhs_T.
            Contracts over D (KT tiles); 1024-wide PSUM tiles, single copy out."""
            for nch in range(T // 1024):
                ps = psG.tile([P, 1024], F32, tag="ps")
                for half in range(2):
                    for kt in range(KT):
                        nc.tensor.matmul(
                            ps[:w_cols, half * 512:(half + 1) * 512],
                            w_sb[:, kt, w_col0:w_col0 + w_cols],
                            rhs_T[:, kt, nch * 1024 + half * 512:nch * 1024 + (half + 1) * 512],
                            start=(kt == 0), stop=(kt == KT - 1))
                if bias_col is None:
                    nc.vector.tensor_copy(out=out_sb[:w_cols, nch * 1024:(nch + 1) * 1024],
                                          in_=ps[:w_cols])
                else:
                    nc.vector.tensor_scalar(out=out_sb[:w_cols, nch * 1024:(nch + 1) * 1024],
                                            in0=ps[:w_cols], scalar1=bias_col,
                                            scalar2=None, op0=mybir.AluOpType.add)

        def mm_val(v_sb, src_T, w_sb, w_col0, bias_b=None):
            """v_sb [P, NB, 130] token-major values (+ones cols) = src.T @ w[:, w_col0:+128]."""
            nc.vector.memset(
                v_sb.rearrange("p n (two c) -> p n two c", two=2)[:, :, :, HD:HD + 1], 1.0)
            for blk in range(NB):
                ps = psG.tile([P, HL], F32, tag="ps")
                for kt in range(KT):
                    nc.tensor.matmul(ps, src_T[:, kt, blk * P:(blk + 1) * P],
                                     w_sb[:, kt, w_col0:w_col0 + HL],
                                     start=(kt == 0), stop=(kt == KT - 1))
                dst = v_sb[:, blk].rearrange("p (two c) -> p two c", two=2)[:, :, :HD]
                src = ps.rearrange("p (two c) -> p two c", two=2)
                if bias_b is None:
                    nc.scalar.copy(out=dst, in_=src)
                else:
                    bb = bias_b.rearrange("p (two c) -> p two c", two=2)
                    nc.vector.tensor_add(out=dst, in0=src, in1=bb)

        def attention(qT, kT, v_sb, attnT_loc, mode, maskT_d, n_kb=NB):
            """attnT_loc [P(2 heads*64), T] bf16 = softmax(qk^T/8, mask) @ v, transposed.
            mode: 'causal' | 'ones' | 'general'. Processes query chunks in pairs
            (1024 queries) so exp runs as one wide ACT instruction."""
            for h in range(2):
                qh = qT[h * HD:(h + 1) * HD]
                kh = kT[h * HD:(h + 1) * HD]
                for qcp in range(NCH // 2):
                    q0 = 2 * qcp           # first 512-chunk of this pair
                    col0 = q0 * 512
                    # keys needed: causal -> kb <= 4*(q0+1)+3
                    kbs = range(min(n_kb, 4 * (q0 + 1) + 4) if mode == "causal" else n_kb)
                    avT = psB.tile([P, 2, 512], F32, tag="avT")
                    for kb in kbs:
                        if mode == "causal":
                            qcs = [qc for qc in (q0, q0 + 1) if 4 * qc + 3 >= kb]
                        else:
                            qcs = [q0, q0 + 1]
                        w = 512 * len(qcs)
                        lo = (qcs[0] - q0) * 512
                        pr = probs_p.tile([P, 1024], BF16, tag="probs")
                        if mode == "general":
                            m_sb = probs_p.tile([P, 1024], BF16, tag="mask")
                            sync.dma_start(
                                out=m_sb,
                                in_=maskT_d.ap()[kb * P:(kb + 1) * P, col0:col0 + 1024])
                        ps = psG.tile([P, 1024], F32, tag="ps")
                        for qc in qcs:
                            nc.tensor.matmul(ps[:, (qc - q0) * 512:(qc - q0 + 1) * 512],
                                             kh[:, kb * P:(kb + 1) * P],
                                             qh[:, qc * 512:(qc + 1) * 512],
                                             start=True, stop=True)
                        nc.scalar.activation(out=pr[:, lo:lo + w], in_=ps[:, lo:lo + w],
                                             func=mybir.ActivationFunctionType.Exp,
                                             scale=float(HD) ** -0.5)
                        if mode == "causal" and kb // 4 in qcs:
                            qc = kb // 4
                            j = kb % 4
                            # keep prob where key k <= query q: (q - k - 128j) >= 0
                            nc.gpsimd.affine_select(
                                out=pr[:, (qc - q0) * 512:(qc - q0 + 1) * 512],
                                in_=pr[:, (qc - q0) * 512:(qc - q0 + 1) * 512],
                                pattern=[[1, 512]], channel_multiplier=-1,
                                base=-128 * j, compare_op=mybir.AluOpType.is_ge,
                                fill=0.0)
                        elif mode == "general":
                            nc.vector.tensor_mul(out=pr[:, lo:lo + w],
                                                 in0=pr[:, lo:lo + w],
                                                 in1=m_sb[:, lo:lo + w])
                        for qc in qcs:
                            last_kb = (min(n_kb - 1, 4 * qc + 3)
                                       if mode == "causal" else n_kb - 1)
                            nc.tensor.matmul(
                                avT[:HD + 1, qc - q0],
                                v_sb[:, kb, h * (HD + 1):(h + 1) * (HD + 1)],
                                pr[:, (qc - q0) * 512:(qc - q0 + 1) * 512],
                                start=(kb == 0), stop=(kb == last_kb))
                    rec = work.tile([HD + 1, 1024], F32, tag="rec", bufs=2)
                    nc.vector.tensor_copy(out=rec[HD:HD + 1],
                                          in_=avT[HD:HD + 1].rearrange("p a b -> p (a b)"))
                    # broadcast the sums row to partitions 0..63 via a DRAM
                    # bounce (step-0 partition APs are only legal on DRAM).
                    rec_d = dram.tile([1, 1024], F32, tag="rec_d", bufs=2)
                    nc.gpsimd.dma_start(out=rec_d, in_=rec[HD:HD + 1])
                    src = bass.AP(tensor=rec_d.tensor, offset=rec_d.offset,
                                  ap=[[0, HD]] + list(rec_d.ap)[1:])
                    nc.gpsimd.dma_start(out=rec[:HD], in_=src)
                    rec2 = work.tile([HD, 1024], F32, tag="rec2", bufs=2)
                    nc.vector.reciprocal(out=rec2, in_=rec[:HD])
                    if DEBUG_TAPS and h == 0:
                        tap_ctr[0] += 1
                        tap(f"sums{tap_ctr[0]}", rec[HD:HD + 1])
                        tap(f"rec{tap_ctr[0]}", rec2)
                    nc.vector.tensor_mul(
                        out=attnT_loc[h * HD:(h + 1) * HD,
                                      col0:col0 + 1024].rearrange("p (a b) -> p a b", a=2),
                        in0=avT[:HD],
                        in1=rec2.rearrange("p (a b) -> p a b", a=2))

        def proj_residual(attnT_full, w_sb, bias_row):
            """x_sb += attnT_full.T @ w (+bias)."""
            for blk in range(NB):
                ps = psG.tile([P, D], F32, tag="ps")
                for kt in range(KT):
                    nc.tensor.matmul(ps, attnT_full[:, kt, blk * P:(blk + 1) * P],
                                     w_sb[:, kt, :], start=(kt == 0), stop=(kt == KT - 1))
                nc.vector.tensor_add(out=x_sb[:, blk], in0=x_sb[:, blk], in1=ps)
                if bias_row is not None:
                    nc.vector.tensor_add(out=x_sb[:, blk], in0=x_sb[:, blk], in1=bias_row)

        # ================ pipeline ================
        xT = big.tile([P, KT, T], BF16, tag="xT", name="x1T")
        layernorm_transpose("ln1", xT)
        tap("x1T", xT)

        qT = big.tile([P, T], BF16, tag="qT", name="qT_self")
        kT = big.tile([P, T], BF16, tag="kT", name="kT_self")
        v_sb = big.tile([P, NB, 2 * (HD + 1)], BF16, tag="v", name="v_self")
        mm_TN(qT, wqkv_sb, 0, HL, xT, qb_sb)
        mm_TN(kT, wqkv_sb, HL, HL, xT, kb_sb)
        mm_val(v_sb, xT, wqkv_sb, 2 * HL, vb_sb)
        tap("qT", qT)
        tap("kT", kT)
        tap("v", v_sb)

        attnT_loc = big.tile([P, T], BF16, tag="attnT", name="attnT_sa")
        attention(qT, kT, v_sb, attnT_loc, self_mode, maskT_self_d)
        tap("attnT_sa", attnT_loc)

        # cross-attention K/V depend only on the encoder; emit them here so the
        # scheduler overlaps their matmuls with the AllGather + projection.
        kcT = big.tile([P, T], BF16, tag="kcT", name="kT_cross")
        vc_sb = big.tile([P, NB, 2 * (HD + 1)], BF16, tag="vc", name="v_cross")
        mm_TN(kcT, wckv_sb, 0, HL, encT_sb, ckb_sb)
        mm_val(vc_sb, encT_sb, wckv_sb, HL, cvb_sb)

        # AllGather self-attention heads (bf16)
        ag1_in = dram.tile([P, T], BF16, name="ag1_in")
        ag1_out = dram.tile([G * P, T], BF16, name="ag1_out")
        nc.gpsimd.dma_start(out=ag1_in, in_=attnT_loc)
        if sim1:
            for r in range(G):
                nc.gpsimd.dma_start(out=ag1_out[r * P:(r + 1) * P, :], in_=ag1_in)
        else:
            nc.gpsimd.collective_compute(
                "AllGather", mybir.AluOpType.bypass,
                replica_groups=[[0, 1, 2, 3], [4, 5, 6, 7]],
                ins=[ag1_in.opt()], outs=[ag1_out.opt()])
        attnT_full = big.tile([P, KT, T], BF16, tag="attnT_full", name="attnT_sa_full")
        sync.dma_start(out=attnT_full,
                       in_=ag1_out.rearrange("(k p) t -> p k t", p=P))
        proj_residual(attnT_full, wsa_sb, sab_sb)
        tap("x_after_sa", x_sb)

        # ---- cross attention ----
        layernorm_transpose("ln2", xT)  # xT now holds x2T
        qcT = big.tile([P, T], BF16, tag="qcT", name="qT_cross")
        mm_TN(qcT, wcq_sb, 0, HL, xT, cqb_sb)
        attnT_ca = big.tile([P, T], BF16, tag="attnT_ca", name="attnT_ca")
        attention(qcT, kcT, vc_sb, attnT_ca, cross_mode, maskT_cross_d, n_kb=S // P)

        ag2_in = dram.tile([P, T], BF16, name="ag2_in")
        ag2_out = dram.tile([G * P, T], BF16, name="ag2_out")
        nc.gpsimd.dma_start(out=ag2_in, in_=attnT_ca)
        if sim1:
            for r in range(G):
                nc.gpsimd.dma_start(out=ag2_out[r * P:(r + 1) * P, :], in_=ag2_in)
        else:
            nc.gpsimd.collective_compute(
                "AllGather", mybir.AluOpType.bypass,
                replica_groups=[[0, 1, 2, 3], [4, 5, 6, 7]],
                ins=[ag2_in.opt()], outs=[ag2_out.opt()])
        sync.dma_start(out=attnT_full,
                       in_=ag2_out.rearrange("(k p) t -> p k t", p=P))
        proj_residual(attnT_full, wca_sb, cab_sb)
        tap("x_after_ca", x_sb)

        # ---- MLP (hidden-slice tensor parallel) ----
        layernorm_transpose("ln3", xT)  # xT now holds x3T
        hT = big.tile([P, HIDL // P, T], BF16, tag="hT", name="hT")
        for hch in range(HIDL // P):
            for nch2 in range(T // 1024):
                ps = psG.tile([P, 1024], F32, tag="ps")
                for half in range(2):
                    c0 = nch2 * 1024 + half * 512
                    for kt in range(KT):
                        nc.tensor.matmul(ps[:, half * 512:(half + 1) * 512],
                                         w1_sb[:, kt, hch * P:(hch + 1) * P],
                                         xT[:, kt, c0:c0 + 512],
                                         start=(kt == 0), stop=(kt == KT - 1))
                nc.scalar.activation(
                    out=hT[:, hch, nch2 * 1024:(nch2 + 1) * 1024], in_=ps,
                    func=mybir.ActivationFunctionType.Gelu,
                    bias=(mb1_cols[:, hch:hch + 1] if b1_nz else 0.0), scale=1.0)

        rs_in = dram.tile([T, D], F32, name="rs_in")
        rs_out = dram.tile([T // G, D], F32, name="rs_out")
        for blk in range(NB):
            ps = psG.tile([P, D], F32, tag="ps")
            for hch in range(HIDL // P):
                nc.tensor.matmul(ps, hT[:, hch, blk * P:(blk + 1) * P],
                                 w2_sb[:, hch, :], start=(hch == 0),
                                 stop=(hch == HIDL // P - 1))
            part = work.tile([P, D], F32, tag="part")
            nc.vector.scalar_tensor_tensor(out=part, in0=x_sb[:, blk], scalar=gate_sb[:, 0:1],
                                           in1=ps, op0=mybir.AluOpType.mult,
                                           op1=mybir.AluOpType.add)
            if b2_nz:
                nc.vector.tensor_add(out=part, in0=part, in1=mb2_sb)
            sync.dma_start(out=rs_in[blk * P:(blk + 1) * P, :], in_=part)

        if sim1:
            nc.gpsimd.dma_start(out=rs_out, in_=rs_in[:T // G, :])
        else:
            nc.gpsimd.collective_compute(
                "ReduceScatter", mybir.AluOpType.add,
                replica_groups=[[0, 1, 2, 3], [4, 5, 6, 7]],
                ins=[rs_in.opt()], outs=[rs_out.opt()])
        sync.dma_start(out=out_d.ap(), in_=rs_out)

        for p in reversed((const, xres, big, work, probs_p, dram, psG, psB)):
            p.release()

    nc.compile()
    return nc


def prepare(inputs):
    """Host-side prep: specialization flags, program build, per-core input maps.
    Returns (nc, in_maps)."""
    x = np.asarray(inputs["x"], np.float32)
    enc = np.asarray(inputs["encoder_out"], np.float32)
    self_mask = np.asarray(inputs["self_mask"]).astype(bool)[0, 0]
    cross_mask = np.asarray(inputs["cross_mask"]).astype(bool)[0, 0]
    qkv_w = np.asarray(inputs["qkv_w"], np.float32)
    qkv_b = np.asarray(inputs["qkv_b"], np.float32)
    sa_proj_w = np.asarray(inputs["sa_proj_w"], np.float32)
    sa_proj_b = np.asarray(inputs["sa_proj_b"], np.float32)
    ln1_g = np.asarray(inputs["ln1_g"], np.float32)
    ln1_b = np.asarray(inputs["ln1_b"], np.float32)
    q_w = np.asarray(inputs["q_w"], np.float32)
    q_b = np.asarray(inputs["q_b"], np.float32)
    kv_w = np.asarray(inputs["kv_w"], np.float32)
    kv_b = np.asarray(inputs["kv_b"], np.float32)
    ca_proj_w = np.asarray(inputs["ca_proj_w"], np.float32)
    ca_proj_b = np.asarray(inputs["ca_proj_b"], np.float32)
    ln2_g = np.asarray(inputs["ln2_g"], np.float32)
    ln2_b = np.asarray(inputs["ln2_b"], np.float32)
    mlp_w1 = np.asarray(inputs["mlp_w1"], np.float32)
    mlp_b1 = np.asarray(inputs["mlp_b1"], np.float32)
    mlp_w2 = np.asarray(inputs["mlp_w2"], np.float32)
    mlp_b2 = np.asarray(inputs["mlp_b2"], np.float32)
    ln3_g = np.asarray(inputs["ln3_g"], np.float32)
    ln3_b = np.asarray(inputs["ln3_b"], np.float32)

    def mask_mode(m):
        if m.all():
            return "ones"
        if np.array_equal(m, np.tril(np.ones(m.shape, bool))):
            return "causal"
        return "general"

    self_mode = mask_mode(self_mask)
    cross_mode = mask_mode(cross_mask)
    if cross_mode == "causal":  # causal path only wired for the self block
        cross_mode = "general"

    def nz(a):
        return bool(np.any(a != 0.0))

    def nontriv(g, b):
        return bool(np.any(g != 1.0) or np.any(b != 0.0))

    cfg = (self_mode, cross_mode,
           nontriv(ln1_g, ln1_b), nontriv(ln2_g, ln2_b), nontriv(ln3_g, ln3_b),
           nz(qkv_b[:D]), nz(qkv_b[D:2 * D]), nz(qkv_b[2 * D:]),
           nz(sa_proj_b), nz(q_b), nz(kv_b[:D]), nz(kv_b[D:]), nz(ca_proj_b),
           nz(mlp_b1), nz(mlp_b2))

    global _last_cfg
    _last_cfg = cfg
    if cfg not in _cache:
        _cache[cfg] = _build(cfg)
    nc = _cache[cfg]

    bf = lambda a: np.ascontiguousarray(a.astype(BF16_NP))
    in_maps = []
    for core in range(8):
        g, r = divmod(core, G)
        hc = slice(r * HL, (r + 1) * HL)       # this core's 128 head columns
        hid = slice(r * HIDL, (r + 1) * HIDL)  # this core's mlp hidden slice
        m = {
            "x": np.ascontiguousarray(x[g]),
            "encT": bf(enc[g].T),
            "wqkv": bf(np.concatenate(
                [qkv_w[:, hc], qkv_w[:, D:][:, hc], qkv_w[:, 2 * D:][:, hc]], axis=1)),
            "wsa": bf(sa_proj_w),
            "wcq": bf(q_w[:, hc]),
            "wckv": bf(np.concatenate([kv_w[:, :D][:, hc], kv_w[:, D:][:, hc]], axis=1)),
            "wca": bf(ca_proj_w),
            "w1": bf(mlp_w1[:, hid]),
            "w2": bf(mlp_w2[hid, :]),
            "gate": np.full((1, 1), 1.0 if r == 0 else 0.0, np.float32),
        }
        if self_mode == "general":
            m["maskT_self"] = bf(self_mask.T.astype(np.float32))
        if cross_mode == "general":
            m["maskT_cross"] = bf(cross_mask.T.astype(np.float32))
        if cfg[2]:
            m["ln1_g"], m["ln1_b"] = ln1_g, ln1_b
        if cfg[3]:
            m["ln2_g"], m["ln2_b"] = ln2_g, ln2_b
        if cfg[4]:
            m["ln3_g"], m["ln3_b"] = ln3_g, ln3_b
        if cfg[5]:
            m["q_bias"] = np.ascontiguousarray(qkv_b[:D][hc])
        if cfg[6]:
            m["k_bias"] = np.ascontiguousarray(qkv_b[D:2 * D][hc])
        if cfg[7]:
            m["v_bias"] = np.ascontiguousarray(qkv_b[2 * D:][hc])
        if cfg[8]:
            m["sa_bias"] = sa_proj_b
        if cfg[9]:
            m["cq_bias"] = np.ascontiguousarray(q_b[hc])
        if cfg[10]:
            m["ck_bias"] = np.ascontiguousarray(kv_b[:D][hc])
        if cfg[11]:
            m["cv_bias"] = np.ascontiguousarray(kv_b[D:][hc])
        if cfg[12]:
            m["ca_bias"] = ca_proj_b
        if cfg[13]:
            m["mlp_b1"] = np.ascontiguousarray(mlp_b1[hid])
        if cfg[14]:
            m["mlp_b2_gated"] = mlp_b2 if r == 0 else np.zeros_like(mlp_b2)
        in_maps.append(m)

    return nc, in_maps


def assemble(results) -> np.ndarray:
    out = np.empty((B, T, D), np.float32)
    for core in range(8):
        g, r = divmod(core, G)
        out[g, r * (T // G):(r + 1) * (T // G), :] = results[core]["out"]
    return out


def kernel(**inputs) -> np.ndarray:
    nc, in_maps = prepare(inputs)
    res = run_bass_kernel_spmd(nc, in_maps, core_ids=list(range(8)))
    global LAST_RESULTS
    LAST_RESULTS = res
    return assemble(res.results)
